# revision 23
# baseline (speedup 1.0000x reference)
"""Trainium2 Bass kernel for the FlowNet-style correlation module.

out[b, u*21+v, i, j] = sum_c x1[b,c,i,j] * x2pad[b,c,i+u,j+v]
with x1, x2: [4, 128, 128, 128] fp32, pad=10, window 21x21 (441 output channels).

Strategy
--------
Sharding: 8 cores = (batch 4) x (H halves). Each core handles one batch's
64-row slab: x1 slice [C=128, 64, 128] and a host-prepadded x2 slice
[C=128, 84, 148] (rows/cols include the +-10 zero halo).

Per core the correlation is computed as 2-D blocked Gram matmuls on the
tensor engine: for each 8x16 pixel block of x1 (M = 128 pixels on PSUM
partitions, C = 128 on the contraction partitions) the moving operand is the
28x36 halo block of x2pad (N = 1008 columns, split into two 504-column PSUM
banks). Inputs are split on the host into fp16 hi + lo parts and each Gram
tile is accumulated as h1.h2 + h1.l2 + l1.h2 — three full-rate fp16 matmuls
whose products are exact in the fp32 PSUM accumulator — giving fp32-level
accuracy at 3 cycles/column (vs 4 for native fp32 matmul).

Each output pixel's 21x21 correlation window is a per-partition band of the
resulting Gram tile [128, 1008]; a per-partition-offset band cannot be
expressed by any on-chip access pattern (and DMA has no PSUM route), so the
device ships the full Gram tiles (2.29x output inflation) and the host
extracts the band while unsharding. The kernel is DMA-bound at ~125us/core
(33MB Gram out + 10MB in at ~360GB/s).
"""

import numpy as np

import concourse.mybir as mybir
import concourse.tile as tile
from concourse import bacc
from concourse.bass_utils import run_bass_kernel_spmd

# Problem constants (hardcoded; kernel.py must be self-contained).
B, C, H, W = 4, 128, 128, 128
PAD = 10
WIN = 21  # correlation window side; WIN**2 = 441 output channels
N_CORES = 8
ROWS = H // 2  # 64 output rows per core
HROWS = ROWS + 2 * PAD  # 84 x2pad rows per core
PW = W + 2 * PAD  # 148 x2pad cols

# 2-D pixel blocking: M-block = DI x DJ = 128 pixels, halo block = NR x NS.
DI, DJ = 8, 16
NR, NS = DI + WIN - 1, DJ + WIN - 1  # 28, 36
NBI, NBJ = ROWS // DI, W // DJ  # 8, 8
NBLK = NBI * NBJ  # 64 blocks per core
NFREE = NR * NS  # 1008 Gram columns per block
RSPLIT = NR // 2  # 14 rows -> 504 columns per matmul (PSUM bank holds 512 fp32)

F32 = mybir.dt.float32

_NC_CACHE = {}


# Tunables (overridable via _build_nc kwargs for experiments).
F16 = mybir.dt.float16
GRAM_BUFS = 6
PSUM_BUFS = 8
DVE_COLS = 360  # columns of each 504-col PSUM tile copied by DVE (rest: ACT)


BI_GROUPS = [(0, 1), (1, 3), (3, 5), (5, 8)]
TRIM_PAD_COLS = False


def _build_nc(gram_bufs=None, psum_bufs=None, dve_cols=None, bi_groups=None, trim=None):
    """fp16 hi/lo 3-pass Gram kernel.

    Each fp32 input is split on the host into fp16 hi + fp16 lo
    (x = h + l + e, |e| <= 2^-23 |x|). The Gram block is accumulated in
    PSUM as h1.h2 + h1.l2 + l1.h2 - three full-rate fp16 matmuls whose
    products are exact in the fp32 PSUM accumulator, giving fp32-level
    accuracy at 3 cycles/column instead of fp32's 4.
    """
    gram_bufs = GRAM_BUFS if gram_bufs is None else gram_bufs
    psum_bufs = PSUM_BUFS if psum_bufs is None else psum_bufs
    dve_cols = DVE_COLS if dve_cols is None else dve_cols
    bi_groups = BI_GROUPS if bi_groups is None else bi_groups
    trim = TRIM_PAD_COLS if trim is None else trim
    key = (gram_bufs, psum_bufs, dve_cols, tuple(bi_groups), trim)
    if key in _NC_CACHE:
        return _NC_CACHE[key]
    nc = bacc.Bacc("TRN2", target_bir_lowering=False, debug=False, num_devices=N_CORES)
    # x1 arrives host-rearranged so each block's 128 pixels are contiguous
    # (the matmul stationary operand AP must have a single free dimension).
    x1hd = nc.dram_tensor("x1h", [C, NBLK, DI * DJ], F16, kind="ExternalInput")
    x1ld = nc.dram_tensor("x1l", [C, NBLK, DI * DJ], F16, kind="ExternalInput")
    x2hd = nc.dram_tensor("x2h", [C, HROWS, PW], F16, kind="ExternalInput")
    x2ld = nc.dram_tensor("x2l", [C, HROWS, PW], F16, kind="ExternalInput")
    gout = nc.dram_tensor("gout", [NBLK, 128, NFREE], F32, kind="ExternalOutput")

    with tile.TileContext(nc) as tc:
        with (
            tc.tile_pool(name="inp", bufs=1) as inp,
            tc.tile_pool(name="gram", bufs=gram_bufs) as gp,
            tc.tile_pool(name="psum", bufs=psum_bufs, space="PSUM") as pp,
        ):
            x1ht = inp.tile([C, NBLK, DI * DJ], F16)
            x1lt = inp.tile([C, NBLK, DI * DJ], F16)
            x2ht = inp.tile([C, HROWS, PW], F16)
            x2lt = inp.tile([C, HROWS, PW], F16)
            # Chunked input loads interleaved (x1 blocks + the x2 rows they
            # need first) so the first matmuls start early instead of
            # waiting for the full input load.
            if trim:
                # The 10 left/right x2 columns are always zero: memset the
                # pad strips once and DMA only the center columns.
                nc.gpsimd.memset(x2ht[:, :, :PAD], 0.0)
                nc.gpsimd.memset(x2ht[:, :, PAD + W :], 0.0)
                nc.gpsimd.memset(x2lt[:, :, :PAD], 0.0)
                nc.gpsimd.memset(x2lt[:, :, PAD + W :], 0.0)
            c0, c1 = (PAD, PAD + W) if trim else (0, PW)
            rprev = 0
            for glo, ghi in bi_groups:
                blo, bhi = glo * NBJ, ghi * NBJ
                rhi = min(HROWS, (ghi - 1) * DI + NR)
                nc.sync.dma_start(x1ht[:, blo:bhi, :], x1hd[:, blo:bhi, :])
                nc.sync.dma_start(
                    x2ht[:, rprev:rhi, c0:c1], x2hd[:, rprev:rhi, c0:c1]
                )
                nc.sync.dma_start(x1lt[:, blo:bhi, :], x1ld[:, blo:bhi, :])
                nc.sync.dma_start(
                    x2lt[:, rprev:rhi, c0:c1], x2ld[:, rprev:rhi, c0:c1]
                )
                rprev = rhi

            for bi in range(NBI):
                i0 = bi * DI
                for bj in range(NBJ):
                    j0 = bj * DJ
                    blk = bi * NBJ + bj
                    g = gp.tile([128, NFREE], F32, tag="g")
                    for h in range(2):
                        ps = pp.tile([128, RSPLIT * NS], F32, tag="ps")
                        r0 = i0 + h * RSPLIT
                        rhsh = x2ht[:, r0 : r0 + RSPLIT, j0 : j0 + NS]
                        rhsl = x2lt[:, r0 : r0 + RSPLIT, j0 : j0 + NS]
                        nc.tensor.matmul(
                            ps[:], x1ht[:, blk, :], rhsh, start=True, stop=False
                        )
                        nc.tensor.matmul(
                            ps[:], x1ht[:, blk, :], rhsl, start=False, stop=False
                        )
                        nc.tensor.matmul(
                            ps[:], x1lt[:, blk, :], rhsh, start=False, stop=True
                        )
                        # Split the PSUM->SBUF copy between DVE and ACT so
                        # neither engine is the bottleneck.
                        base = h * RSPLIT * NS
                        ncol = RSPLIT * NS
                        dcols = min(dve_cols, ncol)
                        nc.vector.tensor_copy(g[:, base : base + dcols], ps[:, :dcols])
                        if dcols < ncol:
                            nc.scalar.copy(
                                g[:, base + dcols : base + ncol], ps[:, dcols:ncol]
                            )
                    nc.sync.dma_start(gout[blk][:], g[:])
    nc.compile()
    _NC_CACHE[key] = nc
    return nc


def _hilo(a):
    h = a.astype(np.float16)
    l = (a - h.astype(np.float32)).astype(np.float16)
    return h, l


def _shard_inputs(x1, x2):
    """Per-core inputs: core k -> batch k//2, row-half k%2 (halo prepadded)."""
    in_maps = []
    for k in range(N_CORES):
        b, half = k // 2, k % 2
        i0 = half * ROWS
        x1s = np.ascontiguousarray(
            x1[b, :, i0 : i0 + ROWS, :]
            .reshape(C, NBI, DI, NBJ, DJ)
            .transpose(0, 1, 3, 2, 4)
            .reshape(C, NBLK, DI * DJ)
        )
        x2s = np.zeros((C, HROWS, PW), dtype=np.float32)
        lo = max(0, PAD - i0)  # first valid padded row
        hi = min(HROWS, H + PAD - i0)  # one past last valid padded row
        x2s[:, lo:hi, PAD : PAD + W] = x2[b, :, i0 - PAD + lo : i0 - PAD + hi, :]
        x1h, x1l = _hilo(x1s)
        x2h, x2l = _hilo(x2s)
        in_maps.append({"x1h": x1h, "x1l": x1l, "x2h": x2h, "x2l": x2l})
    return in_maps


# Band-extraction index arrays (built once).
_IL = np.arange(DI).reshape(DI, 1, 1, 1)
_JL = np.arange(DJ).reshape(1, DJ, 1, 1)
_U = np.arange(WIN).reshape(1, 1, WIN, 1)
_V = np.arange(WIN).reshape(1, 1, 1, WIN)


def _extract_core_output(gout_np):
    """[NBLK, 128, NFREE] Gram tiles -> [441, ROWS, W] correlation output."""
    g = gout_np.reshape(NBI, NBJ, DI, DJ, NR, NS)
    band = g[:, :, _IL, _JL, _IL + _U, _JL + _V]  # (NBI, NBJ, DI, DJ, WIN, WIN)
    # -> (u, v, bi, il, bj, jl) -> (441, ROWS, W)
    return band.transpose(4, 5, 0, 2, 1, 3).reshape(WIN * WIN, ROWS, W)


def kernel(x1: np.ndarray, x2: np.ndarray) -> np.ndarray:
    x1 = np.asarray(x1, dtype=np.float32)
    x2 = np.asarray(x2, dtype=np.float32)
    nc = _build_nc()
    in_maps = _shard_inputs(x1, x2)
    res = run_bass_kernel_spmd(nc, in_maps, core_ids=list(range(N_CORES)))
    out = np.empty((B, WIN * WIN, H, W), dtype=np.float32)
    for k in range(N_CORES):
        b, half = k // 2, k % 2
        i0 = half * ROWS
        out[b, :, i0 : i0 + ROWS, :] = _extract_core_output(res.results[k]["gout"])
    return out


# revision 24
# speedup vs baseline: 1.2706x; 1.2706x over previous
"""Trainium2 Bass kernel for the FlowNet-style correlation module.

out[b, u*21+v, i, j] = sum_c x1[b,c,i,j] * x2pad[b,c,i+u,j+v]
with x1, x2: [4, 128, 128, 128] fp32, pad=10, window 21x21 (441 output channels).

Strategy
--------
Sharding: 8 cores = (batch 4) x (H halves). Each core handles one batch's
64-row slab: x1 slice [C=128, 64, 128] and a host-prepadded x2 slice
[C=128, 84, 148] (rows/cols include the +-10 zero halo).

Per core the correlation is computed as blocked Gram matmuls on the tensor
engine using PE column-tiling: each 4x8 pixel block of x1 (M=32) is a
stationary operand on one 32-column group of the PE array
(tile_position=(0,32g)), and four such blocks run CONCURRENTLY against their
own 24x28 x2pad halo windows (N=672, split into two 336-column PSUM passes).
Hardware-verified: 4 concurrent M=32 col-tiles stream at the same wall time
as a single M=128 matmul, so the small-block shape costs no PE time while
cutting the shipped-Gram inflation from 2.29x (8x16 blocks) to 1.52x.

Inputs are split on the host into fp16 hi + lo parts and each Gram tile is
accumulated as h1.h2 + h1.l2 + l1.h2 - three full-rate fp16 matmuls whose
products are exact in the fp32 PSUM accumulator - giving fp32-level accuracy
(measured 2.9e-07 scale-relative) at 3 cycles/column.

Each output pixel's 21x21 window is a per-partition band of its Gram tile; a
per-partition-offset band cannot be expressed by any on-chip access pattern
(and DMA has no PSUM route), so the device ships the full Gram tiles and the
host extracts the band while unsharding. The kernel is DMA-bound: ~22MB Gram
out + ~10MB in per core at ~360GB/s.
"""

import numpy as np

import concourse.mybir as mybir
import concourse.tile as tile
from concourse import bacc
from concourse.bass_utils import run_bass_kernel_spmd

# Problem constants (hardcoded; kernel.py must be self-contained).
B, C, H, W = 4, 128, 128, 128
PAD = 10
WIN = 21  # correlation window side; WIN**2 = 441 output channels
N_CORES = 8
ROWS = H // 2  # 64 output rows per core
HROWS = ROWS + 2 * PAD  # 84 x2pad rows per core
PW = W + 2 * PAD  # 148 x2pad cols

# Pixel blocking: M-block = DI x DJ = 32 pixels on one PE column group;
# 4 blocks (one quad) run concurrently on the 4 column groups.
DI, DJ = 4, 8
NR, NS = DI + WIN - 1, DJ + WIN - 1  # 24, 28
NBI, NBJ = ROWS // DI, W // DJ  # 16, 16
NQJ = NBJ // 4  # 4 quads per block-row
NQUAD = NBI * NQJ  # 64 quads per core
NFREE = NR * NS  # 672 Gram columns per block
RSPLIT = NR // 2  # 12 rows -> 336 columns per matmul (PSUM bank holds 512 fp32)
NCOL = RSPLIT * NS  # 336

F32 = mybir.dt.float32
F16 = mybir.dt.float16

_NC_CACHE = {}

# Tunables (overridable via _build_nc kwargs for experiments).
GRAM_BUFS = 6
PSUM_BUFS = 8
DVE_COLS = 240  # columns of each 336-col PSUM tile copied by DVE (rest: ACT)
BI_GROUPS = [(0, 2), (2, 6), (6, 11), (11, 16)]


def _build_nc(gram_bufs=None, psum_bufs=None, dve_cols=None, bi_groups=None):
    gram_bufs = GRAM_BUFS if gram_bufs is None else gram_bufs
    psum_bufs = PSUM_BUFS if psum_bufs is None else psum_bufs
    dve_cols = DVE_COLS if dve_cols is None else dve_cols
    bi_groups = BI_GROUPS if bi_groups is None else bi_groups
    key = (gram_bufs, psum_bufs, dve_cols, tuple(bi_groups))
    if key in _NC_CACHE:
        return _NC_CACHE[key]
    nc = bacc.Bacc("TRN2", target_bir_lowering=False, debug=False, num_devices=N_CORES)
    # x1 arrives host-rearranged so each 4x8 block's 32 pixels are contiguous
    # (the matmul stationary operand AP must have a single free dimension).
    NBLK = NBI * NBJ
    x1hd = nc.dram_tensor("x1h", [C, NBLK, DI * DJ], F16, kind="ExternalInput")
    x1ld = nc.dram_tensor("x1l", [C, NBLK, DI * DJ], F16, kind="ExternalInput")
    x2hd = nc.dram_tensor("x2h", [C, HROWS, PW], F16, kind="ExternalInput")
    x2ld = nc.dram_tensor("x2l", [C, HROWS, PW], F16, kind="ExternalInput")
    gout = nc.dram_tensor("gout", [NQUAD, 128, 2 * NCOL], F32, kind="ExternalOutput")

    with tile.TileContext(nc) as tc:
        with (
            tc.tile_pool(name="inp", bufs=1) as inp,
            tc.tile_pool(name="gram", bufs=gram_bufs) as gp,
            tc.tile_pool(name="psum", bufs=psum_bufs, space="PSUM") as pp,
        ):
            x1ht = inp.tile([C, NBLK, DI * DJ], F16)
            x1lt = inp.tile([C, NBLK, DI * DJ], F16)
            x2ht = inp.tile([C, HROWS, PW], F16)
            x2lt = inp.tile([C, HROWS, PW], F16)
            # Chunked input loads (x1 blocks + the x2 rows they need first)
            # so the first matmuls start early.
            rprev = 0
            for glo, ghi in bi_groups:
                blo, bhi = glo * NBJ, ghi * NBJ
                rhi = min(HROWS, (ghi - 1) * DI + NR)
                nc.sync.dma_start(x1ht[:, blo:bhi, :], x1hd[:, blo:bhi, :])
                nc.sync.dma_start(x2ht[:, rprev:rhi, :], x2hd[:, rprev:rhi, :])
                nc.sync.dma_start(x1lt[:, blo:bhi, :], x1ld[:, blo:bhi, :])
                nc.sync.dma_start(x2lt[:, rprev:rhi, :], x2ld[:, rprev:rhi, :])
                rprev = rhi

            for bi in range(NBI):
                i0 = bi * DI
                for qj in range(NQJ):
                    quad = bi * NQJ + qj
                    g = gp.tile([128, 2 * NCOL], F32, tag="g")
                    for h in range(2):
                        ps = pp.tile([128, NCOL], F32, tag="ps")
                        r0 = i0 + h * RSPLIT
                        for grp in range(4):
                            blk = bi * NBJ + qj * 4 + grp
                            j0 = (qj * 4 + grp) * DJ
                            dst = ps[32 * grp : 32 * grp + 32, :]
                            rhsh = x2ht[:, r0 : r0 + RSPLIT, j0 : j0 + NS]
                            rhsl = x2lt[:, r0 : r0 + RSPLIT, j0 : j0 + NS]
                            tp = (0, 32 * grp)
                            nc.tensor.matmul(
                                dst, x1ht[:, blk, :], rhsh,
                                start=True, stop=False,
                                tile_position=tp, skip_group_check=True,
                            )
                            nc.tensor.matmul(
                                dst, x1ht[:, blk, :], rhsl,
                                start=False, stop=False,
                                tile_position=tp, skip_group_check=True,
                            )
                            nc.tensor.matmul(
                                dst, x1lt[:, blk, :], rhsh,
                                start=False, stop=True,
                                tile_position=tp, skip_group_check=True,
                            )
                        # Split the PSUM->SBUF copy between DVE and ACT.
                        base = h * NCOL
                        dcols = min(dve_cols, NCOL)
                        nc.vector.tensor_copy(g[:, base : base + dcols], ps[:, :dcols])
                        if dcols < NCOL:
                            nc.scalar.copy(
                                g[:, base + dcols : base + NCOL], ps[:, dcols:NCOL]
                            )
                    nc.sync.dma_start(gout[quad][:], g[:])
    nc.compile()
    _NC_CACHE[key] = nc
    return nc


def _hilo(a):
    h = a.astype(np.float16)
    l = (a - h.astype(np.float32)).astype(np.float16)
    return h, l


def _shard_inputs(x1, x2):
    """Per-core inputs: core k -> batch k//2, row-half k%2 (halo prepadded)."""
    in_maps = []
    for k in range(N_CORES):
        b, half = k // 2, k % 2
        i0 = half * ROWS
        x1s = np.ascontiguousarray(
            x1[b, :, i0 : i0 + ROWS, :]
            .reshape(C, NBI, DI, NBJ, DJ)
            .transpose(0, 1, 3, 2, 4)
            .reshape(C, NBI * NBJ, DI * DJ)
        )
        x2s = np.zeros((C, HROWS, PW), dtype=np.float32)
        lo = max(0, PAD - i0)  # first valid padded row
        hi = min(HROWS, H + PAD - i0)  # one past last valid padded row
        x2s[:, lo:hi, PAD : PAD + W] = x2[b, :, i0 - PAD + lo : i0 - PAD + hi, :]
        x1h, x1l = _hilo(x1s)
        x2h, x2l = _hilo(x2s)
        in_maps.append({"x1h": x1h, "x1l": x1l, "x2h": x2h, "x2l": x2l})
    return in_maps


# Band-extraction index arrays (built once).  Gram partition p = 32*grp +
# il*DJ + jl; free f = (il+u)*NS + (jl+v).
_G = np.arange(4).reshape(4, 1, 1, 1, 1)
_IL = np.arange(DI).reshape(1, DI, 1, 1, 1)
_JL = np.arange(DJ).reshape(1, 1, DJ, 1, 1)
_U = np.arange(WIN).reshape(1, 1, 1, WIN, 1)
_V = np.arange(WIN).reshape(1, 1, 1, 1, WIN)


def _extract_core_output(gout_np):
    """[NQUAD, 128, 672] Gram tiles -> [441, ROWS, W] correlation output."""
    g = gout_np.reshape(NBI, NQJ, 4, DI, DJ, NR, NS)
    band = g[:, :, _G, _IL, _JL, _IL + _U, _JL + _V]  # (NBI,NQJ,4,DI,DJ,WIN,WIN)
    # -> (u, v, bi, il, qj, grp, jl) -> (441, ROWS, W)
    return band.transpose(5, 6, 0, 3, 1, 2, 4).reshape(WIN * WIN, ROWS, W)


def kernel(x1: np.ndarray, x2: np.ndarray) -> np.ndarray:
    x1 = np.asarray(x1, dtype=np.float32)
    x2 = np.asarray(x2, dtype=np.float32)
    nc = _build_nc()
    in_maps = _shard_inputs(x1, x2)
    res = run_bass_kernel_spmd(nc, in_maps, core_ids=list(range(N_CORES)))
    out = np.empty((B, WIN * WIN, H, W), dtype=np.float32)
    for k in range(N_CORES):
        b, half = k // 2, k % 2
        i0 = half * ROWS
        out[b, :, i0 : i0 + ROWS, :] = _extract_core_output(res.results[k]["gout"])
    return out


# revision 29
# speedup vs baseline: 1.3260x; 1.0436x over previous
"""Trainium2 Bass kernel for the FlowNet-style correlation module.

out[b, u*21+v, i, j] = sum_c x1[b,c,i,j] * x2pad[b,c,i+u,j+v]
with x1, x2: [4, 128, 128, 128] fp32, pad=10, window 21x21 (441 output channels).

Strategy
--------
Sharding: 8 cores = (batch 4) x (H halves). Each core handles one batch's
64-row slab: x1 slice [C=128, 64, 128] and a host-prepadded x2 slice
[C=128, 84, 148] (rows/cols include the +-10 zero halo).

Per core the correlation is computed as blocked Gram matmuls on the tensor
engine using PE column-tiling: each 4x8 pixel block of x1 (M=32) is a
stationary operand on one 32-column group of the PE array
(tile_position=(0,32g)), and four such blocks run CONCURRENTLY against their
own 24x28 x2pad halo windows (N=672, split into two 336-column PSUM passes).
Hardware-verified: 4 concurrent M=32 col-tiles stream at the same wall time
as a single M=128 matmul, so the small-block shape costs no PE time while
cutting the shipped-Gram inflation from 2.29x (8x16 blocks) to 1.52x.

Inputs are split on the host into fp16 hi + lo parts and each Gram tile is
accumulated as h1.h2 + h1.l2 + l1.h2 - three full-rate fp16 matmuls whose
products are exact in the fp32 PSUM accumulator - giving fp32-level accuracy
(measured 2.9e-07 scale-relative) at 3 cycles/column.

Each output pixel's 21x21 window is a per-partition band of its Gram tile; a
per-partition-offset band cannot be expressed by any on-chip access pattern
(and DMA has no PSUM route), so the device ships the full Gram tiles and the
host extracts the band while unsharding. The kernel is DMA-bound: ~22MB Gram
out + ~10MB in per core at ~360GB/s.
"""

import numpy as np

import concourse.mybir as mybir
import concourse.tile as tile
from concourse import bacc
from concourse.bass_utils import run_bass_kernel_spmd

# Problem constants (hardcoded; kernel.py must be self-contained).
B, C, H, W = 4, 128, 128, 128
PAD = 10
WIN = 21  # correlation window side; WIN**2 = 441 output channels
N_CORES = 8
ROWS = H // 2  # 64 output rows per core
HROWS = ROWS + 2 * PAD  # 84 x2pad rows per core
PW = W + 2 * PAD  # 148 x2pad cols

# Pixel blocking: M-block = DI x DJ = 32 pixels on one PE column group;
# 4 blocks (one quad) run concurrently on the 4 column groups.
DI, DJ = 4, 8
NR, NS = DI + WIN - 1, DJ + WIN - 1  # 24, 28
NBI, NBJ = ROWS // DI, W // DJ  # 16, 16
NQJ = NBJ // 4  # 4 quads per block-row
NQUAD = NBI * NQJ  # 64 quads per core
NFREE = NR * NS  # 672 Gram columns per block
RSPLIT = NR // 2  # 12 rows -> 336 columns per matmul (PSUM bank holds 512 fp32)
NCOL = RSPLIT * NS  # 336

F32 = mybir.dt.float32
F16 = mybir.dt.float16

_NC_CACHE = {}

# Tunables (overridable via _build_nc kwargs for experiments).
GRAM_BUFS = 6
PSUM_BUFS = 8
DVE_COLS = 240  # columns of each 336-col PSUM tile copied by DVE (rest: ACT)
BI_GROUPS = [(0, 2), (2, 6), (6, 11), (11, 16)]


QBATCH = 4  # quads per output DMA (1.38MB transfers, above the ~1MB DMA knee)


def _build_nc(
    gram_bufs=None, psum_bufs=None, dve_cols=None, bi_groups=None,
    qbatch=None, passes=3,
):
    gram_bufs = GRAM_BUFS if gram_bufs is None else gram_bufs
    psum_bufs = PSUM_BUFS if psum_bufs is None else psum_bufs
    dve_cols = DVE_COLS if dve_cols is None else dve_cols
    bi_groups = BI_GROUPS if bi_groups is None else bi_groups
    qbatch = QBATCH if qbatch is None else qbatch
    key = (gram_bufs, psum_bufs, dve_cols, tuple(bi_groups), qbatch, passes)
    if key in _NC_CACHE:
        return _NC_CACHE[key]
    nc = bacc.Bacc("TRN2", target_bir_lowering=False, debug=False, num_devices=N_CORES)
    # x1 arrives host-rearranged so each 4x8 block's 32 pixels are contiguous
    # (the matmul stationary operand AP must have a single free dimension).
    NBLK = NBI * NBJ
    x1hd = nc.dram_tensor("x1h", [C, NBLK, DI * DJ], F16, kind="ExternalInput")
    x1ld = nc.dram_tensor("x1l", [C, NBLK, DI * DJ], F16, kind="ExternalInput")
    x2hd = nc.dram_tensor("x2h", [C, HROWS, PW], F16, kind="ExternalInput")
    x2ld = nc.dram_tensor("x2l", [C, HROWS, PW], F16, kind="ExternalInput")
    gout = nc.dram_tensor(
        "gout", [NQUAD // qbatch, 128, qbatch * 2 * NCOL], F32, kind="ExternalOutput"
    )

    with tile.TileContext(nc) as tc:
        with (
            tc.tile_pool(name="inp", bufs=1) as inp,
            tc.tile_pool(name="gram", bufs=gram_bufs) as gp,
            tc.tile_pool(name="psum", bufs=psum_bufs, space="PSUM") as pp,
        ):
            x1ht = inp.tile([C, NBLK, DI * DJ], F16)
            x1lt = inp.tile([C, NBLK, DI * DJ], F16)
            x2ht = inp.tile([C, HROWS, PW], F16)
            x2lt = inp.tile([C, HROWS, PW], F16)
            # Chunked input loads (x1 blocks + the x2 rows they need first)
            # so the first matmuls start early.
            rprev = 0
            for glo, ghi in bi_groups:
                blo, bhi = glo * NBJ, ghi * NBJ
                rhi = min(HROWS, (ghi - 1) * DI + NR)
                nc.sync.dma_start(x1ht[:, blo:bhi, :], x1hd[:, blo:bhi, :])
                nc.sync.dma_start(x2ht[:, rprev:rhi, :], x2hd[:, rprev:rhi, :])
                nc.sync.dma_start(x1lt[:, blo:bhi, :], x1ld[:, blo:bhi, :])
                nc.sync.dma_start(x2lt[:, rprev:rhi, :], x2ld[:, rprev:rhi, :])
                rprev = rhi

            g = None
            for bi in range(NBI):
                i0 = bi * DI
                for qj in range(NQJ):
                    quad = bi * NQJ + qj
                    if quad % qbatch == 0:
                        g = gp.tile([128, qbatch * 2 * NCOL], F32, tag="g")
                    qoff = (quad % qbatch) * 2 * NCOL
                    for h in range(2):
                        ps = pp.tile([128, NCOL], F32, tag="ps")
                        r0 = i0 + h * RSPLIT
                        for grp in range(4):
                            blk = bi * NBJ + qj * 4 + grp
                            j0 = (qj * 4 + grp) * DJ
                            dst = ps[32 * grp : 32 * grp + 32, :]
                            rhsh = x2ht[:, r0 : r0 + RSPLIT, j0 : j0 + NS]
                            rhsl = x2lt[:, r0 : r0 + RSPLIT, j0 : j0 + NS]
                            tp = (0, 32 * grp)
                            nc.tensor.matmul(
                                dst, x1ht[:, blk, :], rhsh,
                                start=True, stop=(passes == 1),
                                tile_position=tp, skip_group_check=True,
                            )
                            if passes == 3:
                                nc.tensor.matmul(
                                    dst, x1ht[:, blk, :], rhsl,
                                    start=False, stop=False,
                                    tile_position=tp, skip_group_check=True,
                                )
                                nc.tensor.matmul(
                                    dst, x1lt[:, blk, :], rhsh,
                                    start=False, stop=True,
                                    tile_position=tp, skip_group_check=True,
                                )
                        # Split the PSUM->SBUF copy between DVE and ACT.
                        base = qoff + h * NCOL
                        dcols = min(dve_cols, NCOL)
                        nc.vector.tensor_copy(g[:, base : base + dcols], ps[:, :dcols])
                        if dcols < NCOL:
                            nc.scalar.copy(
                                g[:, base + dcols : base + NCOL], ps[:, dcols:NCOL]
                            )
                    if quad % qbatch == qbatch - 1:
                        nc.sync.dma_start(gout[quad // qbatch][:], g[:])
    nc.compile()
    _NC_CACHE[key] = nc
    return nc


def _hilo(a):
    h = a.astype(np.float16)
    l = (a - h.astype(np.float32)).astype(np.float16)
    return h, l


def _shard_inputs(x1, x2):
    """Per-core inputs: core k -> batch k//2, row-half k%2 (halo prepadded)."""
    in_maps = []
    for k in range(N_CORES):
        b, half = k // 2, k % 2
        i0 = half * ROWS
        x1s = np.ascontiguousarray(
            x1[b, :, i0 : i0 + ROWS, :]
            .reshape(C, NBI, DI, NBJ, DJ)
            .transpose(0, 1, 3, 2, 4)
            .reshape(C, NBI * NBJ, DI * DJ)
        )
        x2s = np.zeros((C, HROWS, PW), dtype=np.float32)
        lo = max(0, PAD - i0)  # first valid padded row
        hi = min(HROWS, H + PAD - i0)  # one past last valid padded row
        x2s[:, lo:hi, PAD : PAD + W] = x2[b, :, i0 - PAD + lo : i0 - PAD + hi, :]
        x1h, x1l = _hilo(x1s)
        x2h, x2l = _hilo(x2s)
        in_maps.append({"x1h": x1h, "x1l": x1l, "x2h": x2h, "x2l": x2l})
    return in_maps


# Band-extraction index arrays (built once).  Gram partition p = 32*grp +
# il*DJ + jl; free f = (il+u)*NS + (jl+v).
_G = np.arange(4).reshape(4, 1, 1, 1, 1)
_IL = np.arange(DI).reshape(1, DI, 1, 1, 1)
_JL = np.arange(DJ).reshape(1, 1, DJ, 1, 1)
_U = np.arange(WIN).reshape(1, 1, 1, WIN, 1)
_V = np.arange(WIN).reshape(1, 1, 1, 1, WIN)


def _extract_core_output(gout_np):
    """[NQUAD, 128, 672] Gram tiles -> [441, ROWS, W] correlation output."""
    g = gout_np.reshape(NBI, NQJ, 4, DI, DJ, NR, NS)
    band = g[:, :, _G, _IL, _JL, _IL + _U, _JL + _V]  # (NBI,NQJ,4,DI,DJ,WIN,WIN)
    # -> (u, v, bi, il, qj, grp, jl) -> (441, ROWS, W)
    return band.transpose(5, 6, 0, 3, 1, 2, 4).reshape(WIN * WIN, ROWS, W)


def kernel(x1: np.ndarray, x2: np.ndarray) -> np.ndarray:
    x1 = np.asarray(x1, dtype=np.float32)
    x2 = np.asarray(x2, dtype=np.float32)
    nc = _build_nc()
    in_maps = _shard_inputs(x1, x2)
    res = run_bass_kernel_spmd(nc, in_maps, core_ids=list(range(N_CORES)))
    out = np.empty((B, WIN * WIN, H, W), dtype=np.float32)
    for k in range(N_CORES):
        b, half = k // 2, k % 2
        i0 = half * ROWS
        gnp = res.results[k]["gout"]
        if QBATCH > 1:
            gnp = (
                gnp.reshape(NQUAD // QBATCH, 128, QBATCH, 2 * NCOL)
                .transpose(0, 2, 1, 3)
                .reshape(NQUAD, 128, 2 * NCOL)
            )
        out[b, :, i0 : i0 + ROWS, :] = _extract_core_output(gnp)
    return out
